# revision 9
# baseline (speedup 1.0000x reference)
"""Contrastive loss (SimCLR-style) TRN2 Bass kernel, 8-core data-parallel.

Math: z [8192, 256] f32 ->
  zn = z / ||z||row ; S = (zn @ zn.T)/0.1 ; diag masked; row log_softmax;
  loss = -mean_i( S[i, pos(i)] - logsumexp_j S[i, j] ), pos(i) = (i+4096) % 8192.

Strategy: rows sharded 8 ways; each core gets z ROTATED so its own rows sit at
0..1024 (self-diag always in column-block 0, partner diag in block 2 -- one
uniform program, no per-core branching). Per core:
  - SWDGE cast-load z f32->bf16 rows in 5 pipelined groups
  - DVE: x^2 (f16 2x) + log-tree row-reduce -> ss; Quake rsqrt; per-tile
    normalize (4x tensor_scalar) -> zn bf16 scaled by 4 for fp8 headroom
  - SWDGE bounce-write casts bf16->fp8e4 to DRAM; HWDGE X-bar transpose reads
    it back as u16 pairs: znT [128, cols] u16 = fp8 (col, ktile) interleaved
  - PE: DoubleRow fp8 matmuls (K=256 in one instr, 0.5 cyc/col): dense-kt
    weights deinterleaved once (own block cols 0..1024), interleaved ifmap
  - logits drain split: ACT exact Exp(0.625 s - 10) w/ free accum (26/32
    chunks) + DVE Schraudolph magic-add exp (low-u16-as-bf16 view + reduce)
  - self/pos similarities extracted from psum diagonals via identity mask
Host: loss = -mean(0.625 dpos - 10 - log(rs - exp(0.625 sii - 10))).
"""

import numpy as np

N = 8192
K = 256
N_CORES = 8
BLK = N // N_CORES          # 1024 rows per core
MT = BLK // 128             # 8 m-tiles per core
NT = N // 128               # 64 row tiles of full z
GROUP_TILES = [8, 8, 16, 16, 16]   # load/norm groups (row tiles)
CB = 4                      # column blocks of 2048 for matmul/drain
CBW = 2048
ZSCALE = 4.0                # zn pre-scale before fp8 (psum = 16 * cos)
TEMP_INV = 10.0
ASCALE = TEMP_INV / (ZSCALE * ZSCALE)   # 0.625: logits = ASCALE*psum - 10
QMAGIC = 0x5F3759DF
# Schraudolph: exp(ASCALE*s - 10) ~ bf16(u16(A*s + B)); magic add does the
# round+pack: f32 y = A*s + (B + 1.5*2^23), low u16 of y = the bf16 pattern.
SCH_A = float(np.float32((128.0 / np.log(2.0)) * ASCALE))
SCH_B = float(np.float32(16256.0 - 2.0 - (128.0 / np.log(2.0)) * TEMP_INV
                         + 1.5 * 2 ** 23))
# chunk (cb, mt) pairs drained by DVE schraudolph; rest on ACT. cb0 must stay
# on ACT (exact self-term). Spread across cbs/mts for overlap.
DVE_CHUNKS = {(1, 2), (1, 5), (2, 1), (2, 6), (3, 3), (3, 0)}

_CACHE = {}


def _build():
    import concourse.bass as bass
    import concourse.tile as tile
    from concourse import bacc, mybir
    from concourse.bass_interp import get_hw_module

    F32, BF16 = mybir.dt.float32, mybir.dt.bfloat16
    F16, FP8, U16 = mybir.dt.float16, mybir.dt.float8e4, mybir.dt.uint16
    I32 = mybir.dt.int32
    AF, ALU = mybir.ActivationFunctionType, mybir.AluOpType
    AX = mybir.AxisListType

    nc = bacc.Bacc("TRN2", target_bir_lowering=False, debug=False,
                   enable_asserts=False, num_devices=N_CORES)

    zf_in = nc.dram_tensor("zf", [N, K], F32, kind="ExternalInput").ap()
    rs_out = nc.dram_tensor("rs", [128, MT], F32, kind="ExternalOutput").ap()
    ds_out = nc.dram_tensor("ds", [128, MT, 2], F32, kind="ExternalOutput").ap()

    with tile.TileContext(nc) as tc:
        with (
            tc.tile_pool(name="big", bufs=1) as big,
            tc.tile_pool(name="ld", bufs=1) as ldp,
            tc.tile_pool(name="zn", bufs=1) as znp,
            tc.tile_pool(name="sq", bufs=1) as sqp,
            tc.tile_pool(name="tp", bufs=4) as tpp,
            tc.tile_pool(name="wk", bufs=2) as wk,
            tc.tile_pool(name="st", bufs=1) as st,
            tc.tile_pool(name="dt", bufs=1) as dtp,
            tc.tile_pool(name="dram", bufs=1, space=bass.MemorySpace.DRAM) as dram,
            tc.tile_pool(name="ps", bufs=2, space=bass.MemorySpace.PSUM) as psp,
        ):
            magic = st.tile([128, NT], I32)
            nc.vector.memset(magic[:], QMAGIC)
            bias_m10 = st.tile([128, 1], F32)
            nc.vector.memset(bias_m10[:], -TEMP_INV)
            # identity mask [128, 128]: (col == partition)
            iota_j = st.tile([128, 128], I32)
            nc.gpsimd.iota(iota_j[:], pattern=[[1, 128]], base=0,
                           channel_multiplier=0)
            iota_p = st.tile([128, 1], I32)
            nc.gpsimd.iota(iota_p[:], pattern=[[0, 1]], base=0,
                           channel_multiplier=1)
            iota_jf = st.tile([128, 128], F32)
            nc.vector.tensor_copy(iota_jf[:], iota_j[:])
            iota_pf = st.tile([128, 1], F32)
            nc.vector.tensor_copy(iota_pf[:], iota_p[:])
            diagmask = st.tile([128, 128], BF16)
            nc.vector.tensor_scalar(diagmask[:], iota_jf[:], iota_pf[:], None,
                                    op0=ALU.is_equal)

            def rsqrt_dve(ss, nt, tag):
                """rsq = ZSCALE/sqrt(ss): Quake init + 2 Newton + fold scale."""
                ssi = ss[:].bitcast(I32)
                sh = wk.tile([128, nt], I32, tag="sh")
                nc.vector.tensor_scalar(sh[:], ssi, 1, None,
                                        op0=ALU.arith_shift_right)
                y = st.tile([128, nt], F32, tag=f"y_{tag}")
                yi = y[:].bitcast(I32)
                nc.vector.tensor_sub(yi, magic[:, 0:nt], sh[:])
                for it in range(3):
                    y2 = wk.tile([128, nt], F32, tag="nwt")
                    nc.vector.tensor_mul(y2[:], y[:], y[:])
                    xy2 = wk.tile([128, nt], F32, tag="nwt")
                    nc.vector.tensor_mul(xy2[:], ss[:], y2[:])
                    c = wk.tile([128, nt], F32, tag="nwt")
                    last = it == 2
                    nc.vector.tensor_scalar(
                        c[:], xy2[:], -0.5 * (ZSCALE if last else 1.0),
                        1.5 * (ZSCALE if last else 1.0),
                        op0=ALU.mult, op1=ALU.add)
                    yn = st.tile([128, nt], F32, tag=f"y{it}_{tag}")
                    nc.vector.tensor_mul(yn[:], y[:], c[:])
                    y = yn
                return y

            zn8_dram = dram.tile([N, K], FP8)

            def group_chain(g, t0, tpg):
                """load rows -> ss -> rsq -> zn bf16 -> fp8 bounce."""
                zbf = ldp.tile([128, tpg, K], BF16, tag=f"zbf{g}")
                nc.gpsimd.dma_start(
                    zbf[:], zf_in[t0 * 128:(t0 + tpg) * 128, :].rearrange(
                        "(t p) k -> p t k", p=128))
                sq = sqp.tile([128, tpg, K], F16, tag=f"sq{g}")
                nc.vector.tensor_mul(sq[:], zbf[:], zbf[:])
                # log-tree row-reduce in f16 (2x mode), then final reduce
                w = K // 2
                while w >= 16:
                    nc.vector.tensor_tensor(
                        sq[:, :, 0:w], sq[:, :, 0:w], sq[:, :, w:2 * w],
                        op=ALU.add)
                    w //= 2
                ss = st.tile([128, tpg], F32, tag=f"ss{g}")
                nc.vector.reduce_sum(ss[:], sq[:, :, 0:16], axis=AX.X)
                rsq = rsqrt_dve(ss, tpg, f"g{g}")
                zn = znp.tile([128, tpg, K], BF16, tag=f"zn{g}")
                for t in range(tpg):
                    nc.vector.tensor_scalar(zn[:, t, :], zbf[:, t, :],
                                            rsq[:, t:t + 1], None, op0=ALU.mult)
                nc.gpsimd.dma_start(
                    zn8_dram.rearrange("(t p) k -> p t k", p=128)[
                        :, t0:t0 + tpg, :], zn[:])
                return t0 + tpg

            def transpose_read(znT_u16, rows0, nrows, col0):
                """X-bar: zn8 rows [rows0, rows0+nrows) -> znT u16 cols."""
                src = zn8_dram[rows0 * 128:(rows0 + nrows) * 128, :].bitcast(U16)
                nc.sync.dma_start(znT_u16[:, col0:col0 + nrows * 128], src,
                                  transpose=True)

            # schedule: 3 groups ahead, then per-cb matmul+drain
            rs_part = st.tile([128, MT, CB], F32)
            ds_acc = []
            for m in range(MT):
                dt_m = dtp.tile([128, 2, 128], F32, tag=f"dt{m}", name=f"dt{m}")
                ds_acc.append(dt_m)
            wts = big.tile([128, 2, BLK], FP8)

            t0 = 0
            t0 = group_chain(0, t0, GROUP_TILES[0])
            t0 = group_chain(1, t0, GROUP_TILES[1])
            t0 = group_chain(2, t0, GROUP_TILES[2])

            znT_tiles = []
            znT0 = tpp.tile([128, CBW], U16, tag="znT")
            transpose_read(znT0, 0, 8, 0)
            transpose_read(znT0, 8, 8, 1024)
            znT_tiles.append(znT0)

            # weights: deinterleave own block (cols 0..1024) to dense kt-major
            rhs0 = znT0[:].bitcast(FP8).rearrange("p (c k) -> p k c", k=2)
            nc.vector.tensor_copy(wts[:, 0, :], rhs0[:, 0, 0:BLK])
            nc.vector.tensor_copy(wts[:, 1, :], rhs0[:, 1, 0:BLK])

            def do_cb(cb):
                rhs = znT_tiles[cb][:].bitcast(FP8).rearrange(
                    "p (c k) -> p k c", k=2)
                for mt in range(MT):
                    ps = psp.tile([128, CBW], F32, tag="ps")
                    lhsT = wts[:, :, mt * 128:(mt + 1) * 128]
                    for sub in range(CBW // 512):
                        nc.tensor.matmul(
                            ps[:, sub * 512:(sub + 1) * 512], lhsT,
                            rhs[:, :, sub * 512:(sub + 1) * 512],
                            start=True, stop=True,
                            perf_mode=mybir.MatmulPerfMode.DoubleRow)
                    if cb == 0 or cb == 2:
                        j = 0 if cb == 0 else 1
                        nc.vector.tensor_tensor(
                            ds_acc[mt][:, j, :],
                            ps[:, mt * 128:(mt + 1) * 128], diagmask[:],
                            op=ALU.mult)
                    if (cb, mt) in DVE_CHUNKS:
                        sch = wk.tile([128, CBW], F32, tag="sch")
                        nc.vector.tensor_scalar(sch[:], ps[:], SCH_A, SCH_B,
                                                op0=ALU.mult, op1=ALU.add)
                        schbf = sch[:].bitcast(BF16).rearrange(
                            "p (c two) -> p two c", two=2)[:, 0, :]
                        nc.vector.tensor_reduce(
                            rs_part[:, mt, cb:cb + 1], schbf, axis=AX.X,
                            op=ALU.add)
                    else:
                        expo = wk.tile([128, CBW], BF16, tag="expo")
                        nc.scalar.activation(
                            expo[:], ps[:], AF.Exp, bias=bias_m10[:],
                            scale=ASCALE,
                            accum_out=rs_part[:, mt, cb:cb + 1])

            do_cb(0)

            t0 = group_chain(3, t0, GROUP_TILES[3])
            znT1 = tpp.tile([128, CBW], U16, tag="znT")
            transpose_read(znT1, 16, 16, 0)
            znT_tiles.append(znT1)
            do_cb(1)

            t0 = group_chain(4, t0, GROUP_TILES[4])
            znT2 = tpp.tile([128, CBW], U16, tag="znT")
            transpose_read(znT2, 32, 16, 0)
            znT_tiles.append(znT2)
            do_cb(2)

            znT3 = tpp.tile([128, CBW], U16, tag="znT")
            transpose_read(znT3, 48, 16, 0)
            znT_tiles.append(znT3)
            do_cb(3)

            # tails: diag reduces + rowsum fold + outputs
            ds = st.tile([128, MT, 2], F32)
            for mt in range(MT):
                nc.vector.tensor_reduce(ds[:, mt, :], ds_acc[mt][:],
                                        axis=AX.X, op=ALU.add)
            rs_sum = st.tile([128, MT], F32)
            nc.vector.reduce_sum(rs_sum[:], rs_part[:], axis=AX.X)
            nc.sync.dma_start(rs_out, rs_sum[:])
            nc.sync.dma_start(ds_out, ds[:])

    nc.compile()
    nc.m = get_hw_module(nc.m)
    return nc


def _get_nc():
    if "nc" not in _CACHE:
        _CACHE["nc"] = _build()
    return _CACHE["nc"]


def _in_maps(z):
    z = np.ascontiguousarray(z, dtype=np.float32)
    maps = []
    for c in range(N_CORES):
        zc = np.roll(z, -c * BLK, axis=0)
        maps.append({"zf": np.ascontiguousarray(zc)})
    return maps


def _finish(results):
    total = 0.0
    for c in range(N_CORES):
        rs = results[c]["rs"].astype(np.float64)          # [128, MT]
        ds = results[c]["ds"].astype(np.float64)          # [128, MT, 2]
        sii, dpos = ds[:, :, 0], ds[:, :, 1]
        rs_excl = rs - np.exp(ASCALE * sii - TEMP_INV)
        total += (ASCALE * dpos - TEMP_INV - np.log(rs_excl)).sum()
    return np.float32(-total / N)


def kernel(z):
    from concourse import bass_utils
    nc = _get_nc()
    res = bass_utils.run_bass_kernel_spmd(nc, _in_maps(z),
                                          core_ids=list(range(N_CORES)))
    return _finish(res.results)
